# revision 18
# baseline (speedup 1.0000x reference)
"""Distributed MQA attention block (N=2, L=1024, D=4544, H=71, Dkv=64) on 8 TRN2 cores.

Sharding: 2 batch groups x 4-way head tensor-parallel.
  core c = 4*g + j: batch g, heads [18j, 18j+18) (core j=3: 17 real heads + 1 zero pad).
Per core: QKV projection (bf16), RoPE (rotation via PE matmul), causal attention in
S^T = K@Q^T orientation with the softmax sum fused into the V-matmul via 64 appended
ones-columns (sums land in acc rows 64:128, normalized purely on DVE), and a
hand-rolled AllGather: each head-pair's normalized attn^T tile [128, 512] is pushed
to the 3 group peers via single-dest remote_dma_broadcast (SWDGE->SDMA, SBUF->SBUF).
The receiver-side gather layout is XOR-relative (block = 9*(j_me^j_sender) + pair),
and W_dense^T rows are permuted per core on the host to match, so one SPMD program
works for all cores. Dense output is column-sharded; per-q-half dense matmuls are
gated on a remote-arrival semaphore attached post-scheduling (the single-core Tile
scheduling sim cannot observe peer increments). A 1-byte prelude AllGather acts as
the kernel-entry rendezvous so no remote write can race a peer's semaphore reset.
"""

import sys

if "/opt/trn_rl_repo" not in sys.path:
    sys.path.insert(0, "/opt/trn_rl_repo")

import numpy as np
import ml_dtypes

import concourse.bass as bass
import concourse.bacc as bacc
import concourse.mybir as mybir
import concourse.tile as tile
from concourse.bass_utils import run_bass_kernel_spmd

BF16 = mybir.dt.bfloat16
F32 = mybir.dt.float32
AF = mybir.ActivationFunctionType

N, L, D = 2, 1024, 4544
H, DKV = 71, 64
NCORES, GSZ = 8, 4
HPC = 18                 # heads per core (last core of each group: 17 real + 1 pad)
DLOC = HPC * DKV         # 1152
DPAD = GSZ * DLOC        # 4608 = 36 * 128
ESH = D // GSZ           # 1136 output-column shard
NET = 36                 # e-contraction tiles over D=4544 (35 x 128 + 1 x 64)
NMT = DLOC // 128        # 9 m-tiles of Q^T rows (2 heads each)
QB = 512                 # q-block (half of L)
SCALE = 1.0 / np.sqrt(DKV)
REPLICA_GROUPS = [[0, 1, 2, 3], [4, 5, 6, 7]]

_CACHE = {}
DEBUG = False


def _esz(i):
    return 128 if i < NET - 1 else 64


def _emit(tc, nc, io):
    xT, wqkvT, wdT, cosT, sinT, rot, eye, masks2, out = (
        io["xT"], io["wqkvT"], io["wdT"], io["cosT"], io["sinT"], io["rot"],
        io["eye"], io["masks2"], io["out"],
    )
    gsem = io["gsem"]
    lsem = io["lsem"]
    deferred = io["deferred"]  # (inst, sem, val) to gate post-scheduling

    # ---- persistent SBUF (live through the whole kernel) ----
    pers = tc.alloc_tile_pool(name="pers", bufs=1)
    qsb = pers.tile([128, NMT * 1024], BF16, tag="qsb")    # roped Q^T, 2 heads/tile
    ksb = pers.tile([128, 1024], BF16, tag="ksb")          # roped K^T, dup in halves
    vsb = pers.tile([128, 8 * 128], BF16, tag="vsb")       # [V(64) | ones(64)] chunks
    masksb = pers.tile([128, 256], BF16, tag="masksb")     # causal triangle x2 pars
    gath = [pers.tile([128, NET * QB], BF16, tag=f"gath{q}", name=f"gath{q}")
            for q in range(2)]

    nc.sync.dma_start(masksb[:, :], masks2[:, :])
    nc.vector.memset(vsb[:, :], 1.0)  # ones cols survive the V copies below

    # ================= Phase A: QKV projection + RoPE =================
    with (
        tc.tile_pool(name="cst", bufs=1) as cst,
        tc.tile_pool(name="xp", bufs=1) as xp,
        tc.tile_pool(name="wqp", bufs=2) as wqp,
        tc.tile_pool(name="ra", bufs=2) as ra,
        tc.tile_pool(name="rt", bufs=2) as rt,
        tc.tile_pool(name="ps2", bufs=3, space="PSUM") as ps2,
    ):
        cossb = cst.tile([128, 1024], BF16, tag="cossb")
        sinsb = cst.tile([128, 1024], BF16, tag="sinsb")
        rotsb = cst.tile([128, 128], BF16, tag="rotsb")
        eyesb = cst.tile([128, 128], BF16, tag="eyesb")
        nc.sync.dma_start(cossb[:, :], cosT[:, :])
        nc.sync.dma_start(sinsb[:, :], sinT[:, :])
        nc.sync.dma_start(rotsb[:, :], rot[:, :])
        nc.sync.dma_start(eyesb[:, :], eye[:, :])

        xsb = xp.tile([128, NET * 1024], BF16, tag="xsb")
        for i in range(NET):
            p = _esz(i)
            nc.sync.dma_start(xsb[0:p, i * 1024:(i + 1) * 1024],
                              xT[i * 128:i * 128 + p, :])

        def load_w_cols(dst, c0, cw):
            # wqkvT[:, c0:c0+cw] -> dst [128, NET*cw] (tile i at cols i*cw)
            nc.sync.dma_start(
                dst[:, 0:(NET - 1) * cw].rearrange("p (t c) -> p t c", c=cw),
                wqkvT[0:(NET - 1) * 128, c0:c0 + cw].rearrange(
                    "(t p) c -> p t c", p=128),
            )
            nc.sync.dma_start(dst[0:64, (NET - 1) * cw:NET * cw],
                              wqkvT[(NET - 1) * 128:D, c0:c0 + cw])

        # ---- K/V projections (shared KV head), fused: out rows 0:64 = K^T,
        # rows 64:128 = V^T
        wkv = wqp.tile([128, NET * 128], BF16, tag="wm")
        load_w_cols(wkv, DLOC, 128)

        kvps = ps2.tile([128, 1024], F32, tag="ps2")
        for i in range(NET):
            p = _esz(i)
            for q in range(2):
                nc.tensor.matmul(
                    kvps[:, q * QB:(q + 1) * QB],
                    lhsT=wkv[0:p, i * 128:(i + 1) * 128],
                    rhs=xsb[0:p, i * 1024 + q * QB:i * 1024 + (q + 1) * QB],
                    start=(i == 0), stop=(i == NET - 1),
                )
        kvraw = ra.tile([128, 1024], BF16, tag="ra")
        nc.scalar.copy(kvraw[:, :], kvps[:, :])
        # rope K (rows 0:64), then duplicate into rows 64:128 via DMA
        krot = ps2.tile([128, 1024], F32, tag="ps2")
        for q in range(2):
            nc.tensor.matmul(krot[0:64, q * QB:(q + 1) * QB],
                             lhsT=rotsb[0:64, 0:64],
                             rhs=kvraw[0:64, q * QB:(q + 1) * QB],
                             start=True, stop=True)
        for q in range(2):
            s = slice(q * QB, (q + 1) * QB)
            t1 = rt.tile([128, QB], F32, tag="t1")
            t2 = rt.tile([128, QB], F32, tag="t2")
            nc.vector.tensor_mul(t1[0:64, :], kvraw[0:64, s], cossb[0:64, s])
            nc.vector.tensor_mul(t2[0:64, :], krot[0:64, s], sinsb[0:64, s])
            nc.vector.tensor_add(ksb[0:64, s], t1[0:64, :], t2[0:64, :])
        nc.sync.dma_start(ksb[64:128, :], ksb[0:64, :])

        # V^T (kvraw rows 64:128) -> transpose to V [tok, 64] chunks in vsb
        for t8 in range(8):
            vtp = ps2.tile([128, 2048], BF16, tag="ps2")
            nc.tensor.transpose(vtp[0:128, 0:64],
                                kvraw[64:128, t8 * 128:(t8 + 1) * 128],
                                eyesb[64:128, 64:128])
            nc.scalar.copy(vsb[:, t8 * 128:t8 * 128 + 64], vtp[0:128, 0:64])

        # ---- Q projection + RoPE, per m-tile (2 heads each) ----
        for m in range(NMT):
            wqm = wqp.tile([128, NET * 128], BF16, tag="wm")
            load_w_cols(wqm, m * 128, 128)
            qps = ps2.tile([128, 1024], F32, tag="ps2")
            for i in range(NET):
                p = _esz(i)
                for q in range(2):
                    nc.tensor.matmul(
                        qps[:, q * QB:(q + 1) * QB],
                        lhsT=wqm[0:p, i * 128:(i + 1) * 128],
                        rhs=xsb[0:p, i * 1024 + q * QB:i * 1024 + (q + 1) * QB],
                        start=(i == 0), stop=(i == NET - 1),
                    )
            qraw = ra.tile([128, 1024], BF16, tag="ra")
            nc.scalar.copy(qraw[:, :], qps[:, :])
            qrot = ps2.tile([128, 1024], F32, tag="ps2")
            for q in range(2):
                nc.tensor.matmul(qrot[:, q * QB:(q + 1) * QB],
                                 lhsT=rotsb[:, :],
                                 rhs=qraw[:, q * QB:(q + 1) * QB],
                                 start=True, stop=True)
            for q in range(2):
                s = slice(q * QB, (q + 1) * QB)
                t1 = rt.tile([128, QB], F32, tag="t1")
                t2 = rt.tile([128, QB], F32, tag="t2")
                nc.vector.tensor_mul(t1[:, :], qraw[:, s], cossb[:, s])
                nc.vector.tensor_mul(t2[:, :], qrot[:, s], sinsb[:, s])
                nc.vector.tensor_add(qsb[:, m * 1024 + q * QB:m * 1024 + (q + 1) * QB],
                                     t1[:, :], t2[:, :])

    # ================= Phase B: attention + p2p gather + dense =================
    with (
        tc.tile_pool(name="ex", bufs=10) as ex,
        tc.tile_pool(name="at", bufs=4) as at,
        tc.tile_pool(name="rp", bufs=2) as rp,
        tc.tile_pool(name="wdp", bufs=1) as wdp,
        tc.tile_pool(name="op", bufs=2) as op,
        tc.tile_pool(name="sc", bufs=2, space="PSUM") as sc,
        tc.tile_pool(name="ac", bufs=4, space="PSUM") as ac,
    ):
        # dense weights resident; DMAs run in the background during attention
        wdsb = wdp.tile([128, NET * ESH], BF16, tag="wdsb")
        for i in range(NET):
            nc.sync.dma_start(wdsb[:, i * ESH:(i + 1) * ESH],
                              wdT[i * 128:(i + 1) * 128, :])

        def s_burst(qh, hp):
            # scores + exp for both heads of the pair; diagonal k-tile
            # 4*qh+j: columns below 128*j never read downstream (AV clips),
            # the [128j, 128j+128) block gets the triangular mask.
            nkt = 4 * qh + 4
            ess = []
            for kt in range(nkt):
                var = kt - 4 * qh
                c0 = 128 * var if var > 0 else 0
                scp = sc.tile([128, 1024], F32, tag="sc")
                for par in range(2):
                    off = 64 * par
                    nc.tensor.matmul(
                        scp[:, par * QB + c0:(par + 1) * QB],
                        lhsT=ksb[off:off + 64, kt * 128:(kt + 1) * 128],
                        rhs=qsb[off:off + 64,
                                hp * 1024 + qh * QB + c0:hp * 1024 + (qh + 1) * QB],
                        start=True, stop=True,
                    )
                es = ex.tile([128, 1024], BF16, tag="ex")
                if c0 == 0:
                    nc.scalar.activation(es[:, :], scp[:, :], AF.Exp, scale=SCALE)
                else:
                    # one strided activation covering both par halves
                    esv = es[:, :].rearrange("p (t c) -> p t c", c=QB)[:, :, c0:QB]
                    scv = scp[:, :].rearrange("p (t c) -> p t c", c=QB)[:, :, c0:QB]
                    nc.scalar.activation(esv, scv, AF.Exp, scale=SCALE)
                if var >= 0:
                    # causal triangle on the diagonal 128-block, both pars
                    ev = es[:, :].rearrange("p (t c) -> p t c", c=QB)[
                        :, :, c0:c0 + 128]
                    mv = masksb[:, :].rearrange("p (t c) -> p t c", c=128)
                    nc.vector.tensor_mul(ev, ev, mv)
                ess.append((es, c0))
            return ess

        def v_burst(qh, hp, ess):
            # kt-outer so each es tile dies as soon as both pars consumed it
            nkt = 4 * qh + 4
            accs = [ac.tile([128, QB], F32, tag="ac", name=f"acc{i}")
                    for i in range(2)]
            for kt in range(nkt):
                es, c0 = ess[kt]
                for par in range(2):
                    nc.tensor.matmul(
                        accs[par][:, c0:QB],
                        lhsT=vsb[:, kt * 128:(kt + 1) * 128],
                        rhs=es[:, par * QB + c0:(par + 1) * QB],
                        start=(kt == 0), stop=(kt == nkt - 1),
                        skip_group_check=True,
                    )
            return accs

        def dbg_dump_acc(qh, accs):
            for par in range(2):
                dt = op.tile([128, QB], F32, tag="op", name="dbg")
                nc.vector.tensor_copy(dt[:, :], accs[par][:, :])
                nc.sync.dma_start(
                    io["dacc"][:, (2 * qh + par) * QB:(2 * qh + par + 1) * QB],
                    dt[:, :])

        last_mul = [None]

        def normalize_and_send(qh, hp, accs):
            # acc rows 0:64 = attn^T head, rows 64:128 = softmax sums (from the
            # ones columns of vsb). Reciprocal + scale purely on DVE, writing
            # the pair-stacked [128, QB] tile the p2p gather broadcasts.
            atp = at.tile([128, QB], BF16, tag="at")
            pg = 9 * qh + hp  # global pair index = trigger order
            relw = None
            if pg >= 2:
                # the SDMA engines read the at tile and the SWDGE ring entries
                # asynchronously after the trigger; gate on the drain of the
                # send 2 pairs back so (a) at-tile reuse (pool bufs=4) is safe
                # and (b) ring occupancy stays at <= 2*48 = 96 of 128 slots
                relw = nc.vector.wait_ge(lsem, 0)
                # anchor: must not be hoisted before earlier pairs' own muls
                # (whose sends satisfy this very wait)
                if last_mul[0] is not None:
                    tile.add_dep_helper(relw.ins, last_mul[0].ins, sync=False,
                                        reason="drain wait after prev pair")
                deferred.append((relw, lsem, 48 * (pg - 1)))
            for par in range(2):
                acc = accs[par]
                rr = rp.tile([64, QB], F32, tag="rp")
                # plain copy handles the partition crossing; custom-DVE recip
                # runs partition-aligned, in place
                nc.vector.tensor_copy(rr[0:64, :], acc[64:128, :])
                nc.vector.reciprocal_approx_fast(rr[0:64, :], rr[0:64, :])
                mm = nc.vector.tensor_mul(atp[64 * par:64 * par + 64, :],
                                          acc[0:64, :], rr[0:64, :])
                last_mul[0] = mm
                if relw is not None:
                    tile.add_dep_helper(mm.ins, relw.ins, sync=False,
                                        reason="at reuse after send drain")
            # local block (XOR distance 0)
            nc.sync.dma_start(gath[qh][:, hp * QB:(hp + 1) * QB], atp[:, :])
            if DEBUG:
                nc.sync.dma_start(
                    io["datn"][:, hp * 1024 + qh * QB:hp * 1024 + (qh + 1) * QB],
                    atp[:, :])
            # remote blocks: receiver sees this tile at block 9*delta + hp;
            # one SWDGE queue per delta to cut per-ring descriptor backlog
            for d in (1, 2, 3):
                rdests = [None] * 8
                rdests[d] = (0, d)
                nc.gpsimd.remote_dma_broadcast(
                    gath[qh][:, (9 * d + hp) * QB:(9 * d + hp + 1) * QB],
                    atp[:, :],
                    remote_sem=gsem[qh], local_sem=lsem,
                    rdests=rdests, queue_num=0,
                )
            return [nc.gpsimd.trigger_dma(count=None, queue_num=0)]

        def dense(qh, send_insts):
            # gate the whole per-qh dense PE stream on remote arrival:
            # 3 peers x 9 pairs x 2 lanes = 54 increments
            # a real executable wait instruction (a nop would be stripped at
            # lowering, dropping the wait): trivially satisfied >=0 for the
            # scheduling sim, threshold bumped to 54 post-scheduling
            gate = nc.tensor.wait_ge(gsem[qh], 0)
            for si in send_insts:
                tile.add_dep_helper(gate.ins, si.ins, sync=False,
                                    reason="dense gate after sends")
            deferred.append((gate, gsem[qh], 54))
            for m in range(9):
                esz = min(128, ESH - m * 128)
                dps = ac.tile([128, QB], F32, tag="ac")
                for i in range(NET):
                    mm = nc.tensor.matmul(
                        dps[0:esz, :],
                        lhsT=wdsb[:, i * ESH + m * 128:i * ESH + m * 128 + esz],
                        rhs=gath[qh][:, i * QB:(i + 1) * QB],
                        start=(i == 0), stop=(i == NET - 1),
                    )
                    tile.add_dep_helper(mm.ins, gate.ins, sync=False,
                                        reason="dense after arrival gate")
                if DEBUG and m == 0:
                    for i in range(NET):
                        dd = nc.sync.dma_start(
                            io["dgath"][:, (qh * NET + i) * QB:(qh * NET + i + 1) * QB],
                            gath[qh][:, i * QB:(i + 1) * QB])
                        tile.add_dep_helper(dd.ins, gate.ins, sync=True,
                                            reason="dump after gate")
                osb = op.tile([128, QB], F32, tag="op")
                nc.scalar.copy(osb[0:esz, :], dps[0:esz, :])
                nc.sync.dma_start(out[m * 128:m * 128 + esz, qh * QB:(qh + 1) * QB],
                                  osb[0:esz, :])

        # software pipeline over head pairs: S-burst of pair p+1 is emitted
        # before V-burst of pair p, hiding the exp latency
        send_insts = {0: [], 1: []}
        pend = None
        for qh in range(2):
            for hp in range(NMT):
                ess = s_burst(qh, hp)
                if pend is not None:
                    pqh, phm, pess = pend
                    accs = v_burst(pqh, phm, pess)
                    if DEBUG and phm == 0:
                        dbg_dump_acc(pqh, accs)
                        if pqh == 0:
                            for ki in range(2):
                                nc.sync.dma_start(
                                    io["des"][:, ki * 1024:(ki + 1) * 1024],
                                    pess[ki][0][:, :])
                    send_insts[pqh].extend(normalize_and_send(pqh, phm, accs))
                pend = (qh, hp, ess)
            if qh == 0:
                pqh, phm, pess = pend
                send_insts[pqh].extend(
                    normalize_and_send(pqh, phm, v_burst(pqh, phm, pess)))
                pend = None
                dense(0, send_insts[0])
        pqh, phm, pess = pend
        send_insts[pqh].extend(
            normalize_and_send(pqh, phm, v_burst(pqh, phm, pess)))
        dense(1, send_insts[1])
        io["first_trigger_insts"] = send_insts[0][0:1]

    pers.release()


def build():
    if "nc" in _CACHE:
        return _CACHE["nc"]
    nc = bacc.Bacc("TRN2", target_bir_lowering=False, debug=False,
                   num_devices=NCORES)
    gsem = [nc.alloc_semaphore("gsem0"), nc.alloc_semaphore("gsem1")]
    lsem = nc.alloc_semaphore("lsem")
    deferred = []
    io = {
        "xT": nc.dram_tensor("xT", [D, L], BF16, kind="ExternalInput").ap(),
        "wqkvT": nc.dram_tensor("wqkvT", [D, DLOC + 128], BF16,
                                kind="ExternalInput").ap(),
        "wdT": nc.dram_tensor("wdT", [DPAD, ESH], BF16, kind="ExternalInput").ap(),
        "cosT": nc.dram_tensor("cosT", [128, L], BF16, kind="ExternalInput").ap(),
        "sinT": nc.dram_tensor("sinT", [128, L], BF16, kind="ExternalInput").ap(),
        "rot": nc.dram_tensor("rot", [128, 128], BF16, kind="ExternalInput").ap(),
        "eye": nc.dram_tensor("eye", [128, 128], BF16, kind="ExternalInput").ap(),
        "masks2": nc.dram_tensor("masks2", [128, 256], BF16,
                                 kind="ExternalInput").ap(),
        "out": nc.dram_tensor("out", [ESH, L], F32, kind="ExternalOutput").ap(),
        "gsem": gsem, "lsem": lsem, "deferred": deferred,
    }
    if DEBUG:
        io["datn"] = nc.dram_tensor("datn", [128, NMT * 1024], BF16,
                                    kind="ExternalOutput").ap()
        io["dgath"] = nc.dram_tensor("dgath", [128, 2 * NET * QB], BF16,
                                     kind="ExternalOutput").ap()
        io["dacc"] = nc.dram_tensor("dacc", [128, 4 * QB], F32,
                                    kind="ExternalOutput").ap()
        io["des"] = nc.dram_tensor("des", [128, 2 * 1024], BF16,
                                   kind="ExternalOutput").ap()
    with tile.TileContext(nc) as tc:
        _emit(tc, nc, io)

    # post-scheduling: remote-arrival gates (invisible to the scheduling sim)
    for inst, sem, val in deferred:
        inst.wait_op(sem, val, "sem-ge")
    # kernel-entry rendezvous: no remote send until every group peer has
    # entered the kernel (and had its semaphores reset by the preamble)
    nc._bir_kernel_barrier_sem_replica_groups.extend(
        set(g) for g in REPLICA_GROUPS)
    for ft in io["first_trigger_insts"]:
        ft.wait_op(nc._bir_kernel_barrier_sem, nc.bir_kernel_barrier_sem_inc,
                   "sem-ge")
    nc.compile()
    _CACHE["nc"] = nc
    return nc


def make_in_maps(hidden_states, W_qkv, W_dense):
    bf = ml_dtypes.bfloat16
    x = np.asarray(hidden_states, np.float32)
    Wqkv = np.asarray(W_qkv, np.float32)
    Wd = np.asarray(W_dense, np.float32)

    # rope tables, transposed [64, L], replicated to both 64-row halves
    inv = 1.0 / (10000.0 ** (np.arange(0, DKV, 2, dtype=np.float32) / DKV))
    t = np.arange(L, dtype=np.float32)
    freqs = np.outer(t, inv)
    emb = np.concatenate([freqs, freqs], axis=1)          # [L, 64]
    cosT = np.tile(np.cos(emb).T, (2, 1)).astype(bf)      # [128, L]
    sinT = np.tile(np.sin(emb).T, (2, 1)).astype(bf)

    # rotate_half as a matmul: qrot = R1 @ q; lhsT = R1^T; 2-head block diagonal
    R1 = np.zeros((DKV, DKV), np.float32)
    for i in range(32):
        R1[i, i + 32] = -1.0
        R1[i + 32, i] = 1.0
    R2 = np.zeros((128, 128), np.float32)
    R2[:64, :64] = R1
    R2[64:, 64:] = R1
    rot = R2.T.copy().astype(bf)

    eye = np.eye(128, dtype=np.float32).astype(bf)

    # triangular causal mask for the 128x128 diagonal block, twice (2 pars)
    kk = np.arange(128)[:, None]
    qq = np.arange(128)[None, :]
    m1 = (kk <= qq).astype(np.float32)
    masks2 = np.concatenate([m1, m1], axis=1).astype(bf)

    # padded dense weights: W_d^T with 64 zero rows appended (pad head)
    wdT_full = np.concatenate([Wd.T, np.zeros((DPAD - D, D), np.float32)], axis=0)
    wdT_full = wdT_full.astype(bf)

    WkvT = Wqkv[H * DKV:].T.astype(bf)                    # [D, 128]

    in_maps = []
    for c in range(NCORES):
        g, j = divmod(c, GSZ)
        h0 = HPC * j
        nh = HPC if j < GSZ - 1 else H - HPC * (GSZ - 1)  # 18,18,18,17
        WqT = np.zeros((D, DLOC), np.float32)
        WqT[:, :nh * DKV] = Wqkv[DKV * h0:DKV * (h0 + nh)].T
        # dense rows in XOR-relative gather order: block d holds the heads of
        # core (g, j^d)
        wdT_x = np.concatenate(
            [wdT_full[DLOC * (j ^ d):DLOC * ((j ^ d) + 1),
                      ESH * j:ESH * (j + 1)] for d in range(GSZ)], axis=0)
        in_maps.append({
            "xT": np.ascontiguousarray(x[g].T).astype(bf),
            "wqkvT": np.concatenate([WqT.astype(bf), WkvT], axis=1),
            "wdT": np.ascontiguousarray(wdT_x),
            "cosT": cosT, "sinT": sinT, "rot": rot, "eye": eye,
            "masks2": masks2,
        })
    return in_maps


def assemble(results):
    out = np.empty((N, L, D), np.float32)
    for c in range(NCORES):
        g, j = divmod(c, GSZ)
        out[g, :, ESH * j:ESH * (j + 1)] = results[c]["out"].T
    return out


def kernel(hidden_states, W_qkv, W_dense):
    nc = build()
    in_maps = make_in_maps(hidden_states, W_qkv, W_dense)
    res = run_bass_kernel_spmd(nc, in_maps, core_ids=list(range(NCORES)))
    return assemble(res.results)


if __name__ == "__main__":
    import reference
    inputs = reference.setup_inputs()
    out = kernel(**{k: np.asarray(v) for k, v in inputs.items()})
    print("out", out.shape, out.dtype)


# revision 19
# speedup vs baseline: 1.0053x; 1.0053x over previous
"""Distributed MQA attention block (N=2, L=1024, D=4544, H=71, Dkv=64) on 8 TRN2 cores.

Sharding: 2 batch groups x 4-way head tensor-parallel.
  core c = 4*g + j: batch g, heads [18j, 18j+18) (core j=3: 17 real heads + 1 zero pad).
Per core: QKV projection (bf16), RoPE (rotation via PE matmul), causal attention in
S^T = K@Q^T orientation with the softmax sum fused into the V-matmul via 64 appended
ones-columns (sums land in acc rows 64:128, normalized purely on DVE), and a
hand-rolled AllGather: each head-pair's normalized attn^T tile [128, 512] is pushed
to the 3 group peers via single-dest remote_dma_broadcast (SWDGE->SDMA, SBUF->SBUF).
The receiver-side gather layout is XOR-relative (block = 9*(j_me^j_sender) + pair),
and W_dense^T rows are permuted per core on the host to match, so one SPMD program
works for all cores. Dense output is column-sharded; per-q-half dense matmuls are
gated on a remote-arrival semaphore attached post-scheduling (the single-core Tile
scheduling sim cannot observe peer increments). A 1-byte prelude AllGather acts as
the kernel-entry rendezvous so no remote write can race a peer's semaphore reset.
"""

import sys

if "/opt/trn_rl_repo" not in sys.path:
    sys.path.insert(0, "/opt/trn_rl_repo")

import numpy as np
import ml_dtypes

import concourse.bass as bass
import concourse.bacc as bacc
import concourse.mybir as mybir
import concourse.tile as tile
from concourse.bass_utils import run_bass_kernel_spmd

BF16 = mybir.dt.bfloat16
F32 = mybir.dt.float32
AF = mybir.ActivationFunctionType

N, L, D = 2, 1024, 4544
H, DKV = 71, 64
NCORES, GSZ = 8, 4
HPC = 18                 # heads per core (last core of each group: 17 real + 1 pad)
DLOC = HPC * DKV         # 1152
DPAD = GSZ * DLOC        # 4608 = 36 * 128
ESH = D // GSZ           # 1136 output-column shard
NET = 36                 # e-contraction tiles over D=4544 (35 x 128 + 1 x 64)
NMT = DLOC // 128        # 9 m-tiles of Q^T rows (2 heads each)
QB = 512                 # q-block (half of L)
SCALE = 1.0 / np.sqrt(DKV)
REPLICA_GROUPS = [[0, 1, 2, 3], [4, 5, 6, 7]]

_CACHE = {}
DEBUG = False


def _esz(i):
    return 128 if i < NET - 1 else 64


def _emit(tc, nc, io):
    xT, wqkvT, wdT, cosT, sinT, rot, eye, masks2, out = (
        io["xT"], io["wqkvT"], io["wdT"], io["cosT"], io["sinT"], io["rot"],
        io["eye"], io["masks2"], io["out"],
    )
    gsem = io["gsem"]
    lsem = io["lsem"]
    deferred = io["deferred"]  # (inst, sem, val) to gate post-scheduling

    # ---- persistent SBUF (live through the whole kernel) ----
    pers = tc.alloc_tile_pool(name="pers", bufs=1)
    qsb = pers.tile([128, NMT * 1024], BF16, tag="qsb")    # roped Q^T, 2 heads/tile
    ksb = pers.tile([128, 1024], BF16, tag="ksb")          # roped K^T, dup in halves
    vsb = pers.tile([128, 8 * 128], BF16, tag="vsb")       # [V(64) | ones(64)] chunks
    masksb = pers.tile([128, 256], BF16, tag="masksb")     # causal triangle x2 pars
    gath = [pers.tile([128, NET * QB], BF16, tag=f"gath{q}", name=f"gath{q}")
            for q in range(2)]

    nc.sync.dma_start(masksb[:, :], masks2[:, :])
    nc.vector.memset(vsb[:, :], 1.0)  # ones cols survive the V copies below

    # ================= Phase A: QKV projection + RoPE =================
    with (
        tc.tile_pool(name="cst", bufs=1) as cst,
        tc.tile_pool(name="xp", bufs=1) as xp,
        tc.tile_pool(name="wqp", bufs=2) as wqp,
        tc.tile_pool(name="ra", bufs=2) as ra,
        tc.tile_pool(name="rt", bufs=2) as rt,
        tc.tile_pool(name="ps2", bufs=3, space="PSUM") as ps2,
    ):
        cossb = cst.tile([128, 1024], BF16, tag="cossb")
        sinsb = cst.tile([128, 1024], BF16, tag="sinsb")
        rotsb = cst.tile([128, 128], BF16, tag="rotsb")
        eyesb = cst.tile([128, 128], BF16, tag="eyesb")
        nc.sync.dma_start(cossb[:, :], cosT[:, :])
        nc.sync.dma_start(sinsb[:, :], sinT[:, :])
        nc.sync.dma_start(rotsb[:, :], rot[:, :])
        nc.sync.dma_start(eyesb[:, :], eye[:, :])

        xsb = xp.tile([128, NET * 1024], BF16, tag="xsb")
        for i in range(NET):
            p = _esz(i)
            nc.sync.dma_start(xsb[0:p, i * 1024:(i + 1) * 1024],
                              xT[i * 128:i * 128 + p, :])

        def load_w_cols(dst, c0, cw):
            # wqkvT[:, c0:c0+cw] -> dst [128, NET*cw] (tile i at cols i*cw)
            nc.sync.dma_start(
                dst[:, 0:(NET - 1) * cw].rearrange("p (t c) -> p t c", c=cw),
                wqkvT[0:(NET - 1) * 128, c0:c0 + cw].rearrange(
                    "(t p) c -> p t c", p=128),
            )
            nc.sync.dma_start(dst[0:64, (NET - 1) * cw:NET * cw],
                              wqkvT[(NET - 1) * 128:D, c0:c0 + cw])

        # ---- K/V projections (shared KV head), fused: out rows 0:64 = K^T,
        # rows 64:128 = V^T
        wkv = wqp.tile([128, NET * 128], BF16, tag="wm")
        load_w_cols(wkv, DLOC, 128)

        kvps = ps2.tile([128, 1024], F32, tag="ps2")
        for i in range(NET):
            p = _esz(i)
            for q in range(2):
                nc.tensor.matmul(
                    kvps[:, q * QB:(q + 1) * QB],
                    lhsT=wkv[0:p, i * 128:(i + 1) * 128],
                    rhs=xsb[0:p, i * 1024 + q * QB:i * 1024 + (q + 1) * QB],
                    start=(i == 0), stop=(i == NET - 1),
                )
        kvraw = ra.tile([128, 1024], BF16, tag="ra")
        nc.scalar.copy(kvraw[:, :], kvps[:, :])
        # rope K (rows 0:64), then duplicate into rows 64:128 via DMA
        krot = ps2.tile([128, 1024], F32, tag="ps2")
        for q in range(2):
            nc.tensor.matmul(krot[0:64, q * QB:(q + 1) * QB],
                             lhsT=rotsb[0:64, 0:64],
                             rhs=kvraw[0:64, q * QB:(q + 1) * QB],
                             start=True, stop=True)
        for q in range(2):
            s = slice(q * QB, (q + 1) * QB)
            t1 = rt.tile([128, QB], F32, tag="t1")
            t2 = rt.tile([128, QB], F32, tag="t2")
            nc.vector.tensor_mul(t1[0:64, :], kvraw[0:64, s], cossb[0:64, s])
            nc.vector.tensor_mul(t2[0:64, :], krot[0:64, s], sinsb[0:64, s])
            nc.vector.tensor_add(ksb[0:64, s], t1[0:64, :], t2[0:64, :])
        nc.sync.dma_start(ksb[64:128, :], ksb[0:64, :])

        # V^T (kvraw rows 64:128) -> transpose to V [tok, 64] chunks in vsb
        for t8 in range(8):
            vtp = ps2.tile([128, 2048], BF16, tag="ps2")
            nc.tensor.transpose(vtp[0:128, 0:64],
                                kvraw[64:128, t8 * 128:(t8 + 1) * 128],
                                eyesb[64:128, 64:128])
            nc.scalar.copy(vsb[:, t8 * 128:t8 * 128 + 64], vtp[0:128, 0:64])

        # ---- Q projection + RoPE, per m-tile (2 heads each) ----
        for m in range(NMT):
            wqm = wqp.tile([128, NET * 128], BF16, tag="wm")
            load_w_cols(wqm, m * 128, 128)
            qps = ps2.tile([128, 1024], F32, tag="ps2")
            for i in range(NET):
                p = _esz(i)
                for q in range(2):
                    nc.tensor.matmul(
                        qps[:, q * QB:(q + 1) * QB],
                        lhsT=wqm[0:p, i * 128:(i + 1) * 128],
                        rhs=xsb[0:p, i * 1024 + q * QB:i * 1024 + (q + 1) * QB],
                        start=(i == 0), stop=(i == NET - 1),
                    )
            qraw = ra.tile([128, 1024], BF16, tag="ra")
            nc.scalar.copy(qraw[:, :], qps[:, :])
            qrot = ps2.tile([128, 1024], F32, tag="ps2")
            for q in range(2):
                nc.tensor.matmul(qrot[:, q * QB:(q + 1) * QB],
                                 lhsT=rotsb[:, :],
                                 rhs=qraw[:, q * QB:(q + 1) * QB],
                                 start=True, stop=True)
            for q in range(2):
                s = slice(q * QB, (q + 1) * QB)
                t1 = rt.tile([128, QB], F32, tag="t1")
                t2 = rt.tile([128, QB], F32, tag="t2")
                nc.vector.tensor_mul(t1[:, :], qraw[:, s], cossb[:, s])
                nc.vector.tensor_mul(t2[:, :], qrot[:, s], sinsb[:, s])
                nc.vector.tensor_add(qsb[:, m * 1024 + q * QB:m * 1024 + (q + 1) * QB],
                                     t1[:, :], t2[:, :])

    # ================= Phase B: attention + p2p gather + dense =================
    with (
        tc.tile_pool(name="ex", bufs=8) as ex,
        tc.tile_pool(name="at", bufs=9) as at,
        tc.tile_pool(name="rp", bufs=2) as rp,
        tc.tile_pool(name="wdp", bufs=1) as wdp,
        tc.tile_pool(name="op", bufs=2) as op,
        tc.tile_pool(name="sc", bufs=2, space="PSUM") as sc,
        tc.tile_pool(name="ac", bufs=4, space="PSUM") as ac,
    ):
        # dense weights resident; DMAs run in the background during attention
        wdsb = wdp.tile([128, NET * ESH], BF16, tag="wdsb")
        for i in range(NET):
            nc.sync.dma_start(wdsb[:, i * ESH:(i + 1) * ESH],
                              wdT[i * 128:(i + 1) * 128, :])

        def s_burst(qh, hp):
            # scores + exp for both heads of the pair; diagonal k-tile
            # 4*qh+j: columns below 128*j never read downstream (AV clips),
            # the [128j, 128j+128) block gets the triangular mask.
            nkt = 4 * qh + 4
            ess = []
            for kt in range(nkt):
                var = kt - 4 * qh
                c0 = 128 * var if var > 0 else 0
                scp = sc.tile([128, 1024], F32, tag="sc")
                for par in range(2):
                    off = 64 * par
                    nc.tensor.matmul(
                        scp[:, par * QB + c0:(par + 1) * QB],
                        lhsT=ksb[off:off + 64, kt * 128:(kt + 1) * 128],
                        rhs=qsb[off:off + 64,
                                hp * 1024 + qh * QB + c0:hp * 1024 + (qh + 1) * QB],
                        start=True, stop=True,
                    )
                es = ex.tile([128, 1024], BF16, tag="ex")
                if c0 == 0:
                    nc.scalar.activation(es[:, :], scp[:, :], AF.Exp, scale=SCALE)
                else:
                    # one strided activation covering both par halves
                    esv = es[:, :].rearrange("p (t c) -> p t c", c=QB)[:, :, c0:QB]
                    scv = scp[:, :].rearrange("p (t c) -> p t c", c=QB)[:, :, c0:QB]
                    nc.scalar.activation(esv, scv, AF.Exp, scale=SCALE)
                if var >= 0:
                    # causal triangle on the diagonal 128-block, both pars
                    ev = es[:, :].rearrange("p (t c) -> p t c", c=QB)[
                        :, :, c0:c0 + 128]
                    mv = masksb[:, :].rearrange("p (t c) -> p t c", c=128)
                    nc.vector.tensor_mul(ev, ev, mv)
                ess.append((es, c0))
            return ess

        def v_burst(qh, hp, ess):
            # kt-outer so each es tile dies as soon as both pars consumed it
            nkt = 4 * qh + 4
            accs = [ac.tile([128, QB], F32, tag="ac", name=f"acc{i}")
                    for i in range(2)]
            for kt in range(nkt):
                es, c0 = ess[kt]
                for par in range(2):
                    nc.tensor.matmul(
                        accs[par][:, c0:QB],
                        lhsT=vsb[:, kt * 128:(kt + 1) * 128],
                        rhs=es[:, par * QB + c0:(par + 1) * QB],
                        start=(kt == 0), stop=(kt == nkt - 1),
                        skip_group_check=True,
                    )
            return accs

        def dbg_dump_acc(qh, accs):
            for par in range(2):
                dt = op.tile([128, QB], F32, tag="op", name="dbg")
                nc.vector.tensor_copy(dt[:, :], accs[par][:, :])
                nc.sync.dma_start(
                    io["dacc"][:, (2 * qh + par) * QB:(2 * qh + par + 1) * QB],
                    dt[:, :])

        last_mul = [None]
        last_trig = [None]

        def normalize_and_send(qh, hp, accs):
            # acc rows 0:64 = attn^T head, rows 64:128 = softmax sums (from the
            # ones columns of vsb). Reciprocal + scale purely on DVE, writing
            # the pair-stacked [128, QB] tile the p2p gather broadcasts.
            atp = at.tile([128, QB], BF16, tag="at")
            pg = 9 * qh + hp  # global pair index = trigger order
            relw = None
            if pg >= 8:
                # safety net only (at pool bufs=9): the tile being overwritten
                # belongs to pair pg-9 whose sends drained long ago; this wait
                # never binds in practice but guarantees no overwrite race
                relw = nc.vector.wait_ge(lsem, 0)
                # anchor: must not be hoisted before earlier pairs' own muls
                # (whose sends satisfy this very wait)
                if last_mul[0] is not None:
                    tile.add_dep_helper(relw.ins, last_mul[0].ins, sync=False,
                                        reason="drain wait after prev pair")
                deferred.append((relw, lsem, 48 * (pg - 7)))
            for par in range(2):
                acc = accs[par]
                rr = rp.tile([64, QB], F32, tag="rp")
                # plain copy handles the partition crossing; custom-DVE recip
                # runs partition-aligned, in place
                nc.vector.tensor_copy(rr[0:64, :], acc[64:128, :])
                nc.vector.reciprocal_approx_fast(rr[0:64, :], rr[0:64, :])
                mm = nc.vector.tensor_mul(atp[64 * par:64 * par + 64, :],
                                          acc[0:64, :], rr[0:64, :])
                last_mul[0] = mm
                if relw is not None:
                    tile.add_dep_helper(mm.ins, relw.ins, sync=False,
                                        reason="at reuse after send drain")
            # local block (XOR distance 0)
            nc.sync.dma_start(gath[qh][:, hp * QB:(hp + 1) * QB], atp[:, :])
            if DEBUG:
                nc.sync.dma_start(
                    io["datn"][:, hp * 1024 + qh * QB:hp * 1024 + (qh + 1) * QB],
                    atp[:, :])
            # ring-occupancy gate on the gpsimd queue (off the DVE/PE
            # critical path): descriptors of pair pg are generated only after
            # the sends of pair pg-2 drained, so <= 96 of 128 ring slots are
            # ever in flight
            if pg >= 2:
                gw = nc.gpsimd.wait_ge(lsem, 0)
                if last_trig[0] is not None:
                    tile.add_dep_helper(gw.ins, last_trig[0].ins, sync=False,
                                        reason="ring gate after prev trigger")
                deferred.append((gw, lsem, 48 * (pg - 1)))
            # remote blocks: receiver sees this tile at block 9*delta + hp
            for d in (1, 2, 3):
                rdests = [None] * 8
                rdests[d] = (0, d)
                pr = nc.gpsimd.remote_dma_broadcast(
                    gath[qh][:, (9 * d + hp) * QB:(9 * d + hp + 1) * QB],
                    atp[:, :],
                    remote_sem=gsem[qh], local_sem=lsem,
                    rdests=rdests, queue_num=0,
                )
                if pg >= 2:
                    tile.add_dep_helper(pr.ins, gw.ins, sync=False,
                                        reason="preps after ring gate")
            tr = nc.gpsimd.trigger_dma(count=None, queue_num=0)
            last_trig[0] = tr
            return [tr]

        def dense(qh, send_insts):
            # gate the whole per-qh dense PE stream on remote arrival:
            # 3 peers x 9 pairs x 2 lanes = 54 increments
            # a real executable wait instruction (a nop would be stripped at
            # lowering, dropping the wait): trivially satisfied >=0 for the
            # scheduling sim, threshold bumped to 54 post-scheduling
            gate = nc.tensor.wait_ge(gsem[qh], 0)
            for si in send_insts:
                tile.add_dep_helper(gate.ins, si.ins, sync=False,
                                    reason="dense gate after sends")
            deferred.append((gate, gsem[qh], 54))
            for m in range(9):
                esz = min(128, ESH - m * 128)
                dps = ac.tile([128, QB], F32, tag="ac")
                for i in range(NET):
                    mm = nc.tensor.matmul(
                        dps[0:esz, :],
                        lhsT=wdsb[:, i * ESH + m * 128:i * ESH + m * 128 + esz],
                        rhs=gath[qh][:, i * QB:(i + 1) * QB],
                        start=(i == 0), stop=(i == NET - 1),
                    )
                    tile.add_dep_helper(mm.ins, gate.ins, sync=False,
                                        reason="dense after arrival gate")
                if DEBUG and m == 0:
                    for i in range(NET):
                        dd = nc.sync.dma_start(
                            io["dgath"][:, (qh * NET + i) * QB:(qh * NET + i + 1) * QB],
                            gath[qh][:, i * QB:(i + 1) * QB])
                        tile.add_dep_helper(dd.ins, gate.ins, sync=True,
                                            reason="dump after gate")
                osb = op.tile([128, QB], F32, tag="op")
                nc.scalar.copy(osb[0:esz, :], dps[0:esz, :])
                nc.sync.dma_start(out[m * 128:m * 128 + esz, qh * QB:(qh + 1) * QB],
                                  osb[0:esz, :])

        # software pipeline over head pairs: S-burst of pair p+1 is emitted
        # before V-burst of pair p, hiding the exp latency
        send_insts = {0: [], 1: []}
        pend = None
        for qh in range(2):
            for hp in range(NMT):
                ess = s_burst(qh, hp)
                if pend is not None:
                    pqh, phm, pess = pend
                    accs = v_burst(pqh, phm, pess)
                    if DEBUG and phm == 0:
                        dbg_dump_acc(pqh, accs)
                        if pqh == 0:
                            for ki in range(2):
                                nc.sync.dma_start(
                                    io["des"][:, ki * 1024:(ki + 1) * 1024],
                                    pess[ki][0][:, :])
                    send_insts[pqh].extend(normalize_and_send(pqh, phm, accs))
                pend = (qh, hp, ess)
            if qh == 0:
                pqh, phm, pess = pend
                send_insts[pqh].extend(
                    normalize_and_send(pqh, phm, v_burst(pqh, phm, pess)))
                pend = None
                dense(0, send_insts[0])
        pqh, phm, pess = pend
        send_insts[pqh].extend(
            normalize_and_send(pqh, phm, v_burst(pqh, phm, pess)))
        dense(1, send_insts[1])
        io["first_trigger_insts"] = send_insts[0][0:1]

    pers.release()


def build():
    if "nc" in _CACHE:
        return _CACHE["nc"]
    nc = bacc.Bacc("TRN2", target_bir_lowering=False, debug=False,
                   num_devices=NCORES)
    gsem = [nc.alloc_semaphore("gsem0"), nc.alloc_semaphore("gsem1")]
    lsem = nc.alloc_semaphore("lsem")
    deferred = []
    io = {
        "xT": nc.dram_tensor("xT", [D, L], BF16, kind="ExternalInput").ap(),
        "wqkvT": nc.dram_tensor("wqkvT", [D, DLOC + 128], BF16,
                                kind="ExternalInput").ap(),
        "wdT": nc.dram_tensor("wdT", [DPAD, ESH], BF16, kind="ExternalInput").ap(),
        "cosT": nc.dram_tensor("cosT", [128, L], BF16, kind="ExternalInput").ap(),
        "sinT": nc.dram_tensor("sinT", [128, L], BF16, kind="ExternalInput").ap(),
        "rot": nc.dram_tensor("rot", [128, 128], BF16, kind="ExternalInput").ap(),
        "eye": nc.dram_tensor("eye", [128, 128], BF16, kind="ExternalInput").ap(),
        "masks2": nc.dram_tensor("masks2", [128, 256], BF16,
                                 kind="ExternalInput").ap(),
        "out": nc.dram_tensor("out", [ESH, L], F32, kind="ExternalOutput").ap(),
        "gsem": gsem, "lsem": lsem, "deferred": deferred,
    }
    if DEBUG:
        io["datn"] = nc.dram_tensor("datn", [128, NMT * 1024], BF16,
                                    kind="ExternalOutput").ap()
        io["dgath"] = nc.dram_tensor("dgath", [128, 2 * NET * QB], BF16,
                                     kind="ExternalOutput").ap()
        io["dacc"] = nc.dram_tensor("dacc", [128, 4 * QB], F32,
                                    kind="ExternalOutput").ap()
        io["des"] = nc.dram_tensor("des", [128, 2 * 1024], BF16,
                                   kind="ExternalOutput").ap()
    with tile.TileContext(nc) as tc:
        _emit(tc, nc, io)

    # post-scheduling: remote-arrival gates (invisible to the scheduling sim)
    for inst, sem, val in deferred:
        inst.wait_op(sem, val, "sem-ge")
    # kernel-entry rendezvous: no remote send until every group peer has
    # entered the kernel (and had its semaphores reset by the preamble)
    nc._bir_kernel_barrier_sem_replica_groups.extend(
        set(g) for g in REPLICA_GROUPS)
    for ft in io["first_trigger_insts"]:
        ft.wait_op(nc._bir_kernel_barrier_sem, nc.bir_kernel_barrier_sem_inc,
                   "sem-ge")
    nc.compile()
    _CACHE["nc"] = nc
    return nc


def make_in_maps(hidden_states, W_qkv, W_dense):
    bf = ml_dtypes.bfloat16
    x = np.asarray(hidden_states, np.float32)
    Wqkv = np.asarray(W_qkv, np.float32)
    Wd = np.asarray(W_dense, np.float32)

    # rope tables, transposed [64, L], replicated to both 64-row halves
    inv = 1.0 / (10000.0 ** (np.arange(0, DKV, 2, dtype=np.float32) / DKV))
    t = np.arange(L, dtype=np.float32)
    freqs = np.outer(t, inv)
    emb = np.concatenate([freqs, freqs], axis=1)          # [L, 64]
    cosT = np.tile(np.cos(emb).T, (2, 1)).astype(bf)      # [128, L]
    sinT = np.tile(np.sin(emb).T, (2, 1)).astype(bf)

    # rotate_half as a matmul: qrot = R1 @ q; lhsT = R1^T; 2-head block diagonal
    R1 = np.zeros((DKV, DKV), np.float32)
    for i in range(32):
        R1[i, i + 32] = -1.0
        R1[i + 32, i] = 1.0
    R2 = np.zeros((128, 128), np.float32)
    R2[:64, :64] = R1
    R2[64:, 64:] = R1
    rot = R2.T.copy().astype(bf)

    eye = np.eye(128, dtype=np.float32).astype(bf)

    # triangular causal mask for the 128x128 diagonal block, twice (2 pars)
    kk = np.arange(128)[:, None]
    qq = np.arange(128)[None, :]
    m1 = (kk <= qq).astype(np.float32)
    masks2 = np.concatenate([m1, m1], axis=1).astype(bf)

    # padded dense weights: W_d^T with 64 zero rows appended (pad head)
    wdT_full = np.concatenate([Wd.T, np.zeros((DPAD - D, D), np.float32)], axis=0)
    wdT_full = wdT_full.astype(bf)

    WkvT = Wqkv[H * DKV:].T.astype(bf)                    # [D, 128]

    in_maps = []
    for c in range(NCORES):
        g, j = divmod(c, GSZ)
        h0 = HPC * j
        nh = HPC if j < GSZ - 1 else H - HPC * (GSZ - 1)  # 18,18,18,17
        WqT = np.zeros((D, DLOC), np.float32)
        WqT[:, :nh * DKV] = Wqkv[DKV * h0:DKV * (h0 + nh)].T
        # dense rows in XOR-relative gather order: block d holds the heads of
        # core (g, j^d)
        wdT_x = np.concatenate(
            [wdT_full[DLOC * (j ^ d):DLOC * ((j ^ d) + 1),
                      ESH * j:ESH * (j + 1)] for d in range(GSZ)], axis=0)
        in_maps.append({
            "xT": np.ascontiguousarray(x[g].T).astype(bf),
            "wqkvT": np.concatenate([WqT.astype(bf), WkvT], axis=1),
            "wdT": np.ascontiguousarray(wdT_x),
            "cosT": cosT, "sinT": sinT, "rot": rot, "eye": eye,
            "masks2": masks2,
        })
    return in_maps


def assemble(results):
    out = np.empty((N, L, D), np.float32)
    for c in range(NCORES):
        g, j = divmod(c, GSZ)
        out[g, :, ESH * j:ESH * (j + 1)] = results[c]["out"].T
    return out


def kernel(hidden_states, W_qkv, W_dense):
    nc = build()
    in_maps = make_in_maps(hidden_states, W_qkv, W_dense)
    res = run_bass_kernel_spmd(nc, in_maps, core_ids=list(range(NCORES)))
    return assemble(res.results)


if __name__ == "__main__":
    import reference
    inputs = reference.setup_inputs()
    out = kernel(**{k: np.asarray(v) for k, v in inputs.items()})
    print("out", out.shape, out.dtype)
